# revision 1
# baseline (speedup 1.0000x reference)
"""MinGRU Trainium2 kernel.

Problem: nn_MinGRU (B=8, T=4096, D=1024, fp32)
    k  = h @ W_z.T + b_z
    th = h @ W_h.T + b_h
    h[t] = (1-z[t]) * h[t-1] + z[t]*g(th[t]),  z = sigmoid(k)
    g(x) = x+0.5 for x>=0 else sigmoid(x)   (the reference computes this
    recurrence in log space; we use the mathematically identical linear-space
    form, which is stable since 0 < 1-z < 1):
    a[t] = sigmoid(-k[t]) = 1 - z[t]
    b[t] = z[t] * g(th[t]),   g(x) = max(x + 0.5, sigmoid(x))
    h[t] = a[t]*h[t-1] + b[t]   -> VectorE tensor_tensor_scan (fp32 state)

Sharding: data-parallel over batch — core i processes sample i ([T, D]).
Weights replicated; host pre-transposes them to [d, e] (matmul lhsT layout).

Per-core dataflow ([e,t] layout so the scan runs along the free dim):
  h --SWDGE cast DMA (fp32->bf16)--> h_nat [t,d] --DMA xbar--> hT [d,t]
  bf16 matmuls (fp32 PSUM accumulate), sigmoids on ScalarE from PSUM,
  gating algebra on VectorE/GpSimd, recurrence via tensor_tensor_scan
  (fp32 state, bf16 output), PE bf16 transposes back to [t,e],
  ScalarE copy (bf16->fp32) into the output staging tile, fp32 DMA store.
"""

import contextlib
import numpy as np
import concourse.bass as bass
import concourse.bacc as bacc
import concourse.mybir as mybir
import concourse.tile as tile
from concourse.bass_utils import run_bass_kernel_spmd
from concourse.masks import make_identity

F32 = mybir.dt.float32
BF16 = mybir.dt.bfloat16
AF = mybir.ActivationFunctionType
OP = mybir.AluOpType

B, T, D = 8, 4096, 1024
NC_CORES = 8
TC = 512                 # time chunk (one fp32 PSUM bank)
NCHUNK = T // TC         # 8
NE = D // 128            # 8 e-tiles
ND = D // 128            # 8 d-tiles
NTB = TC // 128          # 4 t-blocks per chunk


def build_program():
    nc = bacc.Bacc("TRN2", target_bir_lowering=False, debug=False)
    h_d = nc.dram_tensor("h", [T, D], F32, kind="ExternalInput").ap()
    wzT_d = nc.dram_tensor("wzT", [D, D], F32, kind="ExternalInput").ap()
    whT_d = nc.dram_tensor("whT", [D, D], F32, kind="ExternalInput").ap()
    bz_d = nc.dram_tensor("bz", [128, NE], F32, kind="ExternalInput").ap()
    bh_d = nc.dram_tensor("bh", [128, NE], F32, kind="ExternalInput").ap()
    out_d = nc.dram_tensor("out", [T, D], F32, kind="ExternalOutput").ap()

    with tile.TileContext(nc) as tc, contextlib.ExitStack() as ctx:
        const = ctx.enter_context(tc.tile_pool(name="const", bufs=1))
        hnatp = ctx.enter_context(tc.tile_pool(name="hnat", bufs=2))
        hTp = ctx.enter_context(tc.tile_pool(name="hT", bufs=2))
        mmps = ctx.enter_context(tc.tile_pool(name="mmps", bufs=3, space="PSUM"))
        trps = ctx.enter_context(tc.tile_pool(name="trps", bufs=2, space="PSUM"))
        ew = ctx.enter_context(tc.tile_pool(name="ew", bufs=2))
        osbp = ctx.enter_context(tc.tile_pool(name="osb", bufs=2))
        hbp = ctx.enter_context(tc.tile_pool(name="hb", bufs=2))

        # ---- constants ----
        wz_sb = const.tile([128, ND, D], BF16)   # [d%128, d_tile, e]
        wh_sb = const.tile([128, ND, D], BF16)
        bz_sb = const.tile([128, NE], F32)
        bh_sb = const.tile([128, NE], F32)
        nc.sync.dma_start(bz_sb, bz_d)
        nc.sync.dma_start(bh_sb, bh_d)
        negbz = const.tile([128, NE], F32)
        bh05 = const.tile([128, NE], F32)
        nc.gpsimd.tensor_scalar_mul(negbz, bz_sb, -1.0)
        nc.gpsimd.tensor_scalar_add(bh05, bh_sb, 0.5)
        ident = const.tile([128, 128], F32)
        make_identity(nc, ident)
        ident_bf = const.tile([128, 128], BF16)
        nc.gpsimd.tensor_copy(ident_bf, ident)

        hT_tiles = {}

        def load_and_transpose_chunk(ci):
            # cast-load (fp32->bf16, SWDGE) h chunk in natural [t, d] layout,
            # then transpose to [d, t] via the DMA xbar
            h_nat = hnatp.tile([128, NTB, D], BF16, name=f"h_nat{ci}",
                               tag="h_nat")
            hsrc = bass.AP(
                tensor=h_d.tensor,
                offset=h_d.offset + ci * TC * D,
                ap=[[D, 128], [128 * D, NTB], [1, D]],
            )
            nc.gpsimd.dma_start(h_nat, hsrc)
            hT = hTp.tile([128, ND, TC], BF16, name=f"hT{ci}", tag="hT")
            for tb in range(NTB):
                nc.sync.dma_start(
                    hT[:, :, tb * 128:(tb + 1) * 128],
                    h_nat[:, tb, :],
                    transpose=True,
                )
            hT_tiles[ci] = hT

        # cast fp32->bf16 during DMA (SWDGE)
        nc.gpsimd.dma_start(wz_sb, wzT_d.rearrange("(dt p) e -> p dt e", p=128))
        nc.gpsimd.dma_start(wh_sb, whT_d.rearrange("(dt p) e -> p dt e", p=128))
        load_and_transpose_chunk(0)

        prev_hb = [None] * NE

        for tci in range(NCHUNK):
            hT = hT_tiles.pop(tci)
            if tci + 1 < NCHUNK:
                load_and_transpose_chunk(tci + 1)

            out_sb = osbp.tile([128, NTB, D], F32, name=f"out_sb{tci}",
                               tag="out_sb")

            # Phase 1: all matmuls of the chunk (dense PE stream)
            kk, tt = [], []
            for e in range(NE):
                es = slice(e * 128, (e + 1) * 128)
                k_ps = mmps.tile([128, TC], F32, name=f"k{tci}_{e}", tag="k")
                th_ps = mmps.tile([128, TC], F32, name=f"th{tci}_{e}", tag="th")
                for d in range(ND):
                    nc.tensor.matmul(k_ps, wz_sb[:, d, es], hT[:, d, :],
                                     start=(d == 0), stop=(d == ND - 1))
                for d in range(ND):
                    nc.tensor.matmul(th_ps, wh_sb[:, d, es], hT[:, d, :],
                                     start=(d == 0), stop=(d == ND - 1))
                kk.append(k_ps)
                tt.append(th_ps)

            # Phase 2: pointwise + scan per e-tile
            hbs = []
            for e in range(NE):
                k_ps, th_ps = kk[e], tt[e]
                # z = sigmoid(k + bz); s = sigmoid(th + bh)
                z_t = ew.tile([128, TC], F32, name=f"z{tci}_{e}", tag="z")
                s_t = ew.tile([128, TC], F32, name=f"s{tci}_{e}", tag="s")
                nc.scalar.activation(s_t, th_ps, AF.Sigmoid,
                                     bias=bh_sb[:, e:e + 1])
                nc.scalar.activation(z_t, k_ps, AF.Sigmoid,
                                     bias=bz_sb[:, e:e + 1])
                # a = 1 - z  (VectorE: (z - 1) * -1)
                a_t = ew.tile([128, TC], F32, name=f"a{tci}_{e}", tag="a")
                nc.vector.tensor_scalar(a_t, z_t, 1.0, -1.0,
                                        op0=OP.subtract, op1=OP.mult)
                # g = max(th + bh + 0.5, s)
                g_t = ew.tile([128, TC], F32, name=f"g{tci}_{e}", tag="g")
                nc.vector.scalar_tensor_tensor(g_t, th_ps, bh05[:, e:e + 1],
                                               s_t, op0=OP.add, op1=OP.max)
                # b = z * g
                b_t = ew.tile([128, TC], F32, name=f"b{tci}_{e}", tag="b")
                nc.gpsimd.tensor_tensor(b_t, z_t, g_t, OP.mult)
                # h[t] = a[t]*h[t-1] + b[t]; fp32 state, bf16 output
                hb = hbp.tile([128, TC], BF16, name=f"hb{tci}_{e}", tag=f"hb{e}")
                init = 0.0 if tci == 0 else prev_hb[e][:, TC - 1:TC]
                nc.vector.tensor_tensor_scan(hb, a_t, b_t, init,
                                             OP.mult, OP.add)
                prev_hb[e] = hb
                hbs.append(hb)

            # Phase 3: PE bf16 transposes back to [t, e] + cast-assemble
            for e in range(NE):
                es = slice(e * 128, (e + 1) * 128)
                tr_ps = trps.tile([128, NTB, 128], BF16, name=f"tr{tci}_{e}",
                                  tag="tr")
                for tb in range(NTB):
                    nc.tensor.transpose(tr_ps[:, tb, :],
                                        hbs[e][:, tb * 128:(tb + 1) * 128],
                                        ident_bf)
                nc.scalar.copy(out_sb[:, :, es], tr_ps)  # bf16 -> fp32

            # ---- store chunk (plain fp32 HWDGE) ----
            dst = bass.AP(
                tensor=out_d.tensor,
                offset=out_d.offset + tci * TC * D,
                ap=[[D, 128], [128 * D, NTB], [1, D]],
            )
            nc.sync.dma_start(dst, out_sb)

    nc.compile()
    return nc


_nc_cache = None


def _get_program():
    global _nc_cache
    if _nc_cache is None:
        _nc_cache = build_program()
    return _nc_cache


def _make_in_maps(h_prev_layer, W_z, b_z, W_h, b_h):
    wzT = np.ascontiguousarray(W_z.T.astype(np.float32))
    whT = np.ascontiguousarray(W_h.T.astype(np.float32))
    bz8 = np.ascontiguousarray(b_z.reshape(NE, 128).T.astype(np.float32))
    bh8 = np.ascontiguousarray(b_h.reshape(NE, 128).T.astype(np.float32))
    return [
        {
            "h": np.ascontiguousarray(h_prev_layer[i].astype(np.float32)),
            "wzT": wzT, "whT": whT, "bz": bz8, "bh": bh8,
        }
        for i in range(B)
    ]


def run(inputs, trace=False, **kw):
    nc = _get_program()
    in_maps = _make_in_maps(**inputs)
    res = run_bass_kernel_spmd(nc, in_maps, core_ids=list(range(NC_CORES)),
                               trace=trace, **kw)
    out = np.stack([res.results[i]["out"] for i in range(NC_CORES)], axis=0)
    return out, res


def kernel(h_prev_layer, W_z, b_z, W_h, b_h):
    out, _ = run(dict(h_prev_layer=h_prev_layer, W_z=W_z, b_z=b_z,
                      W_h=W_h, b_h=b_h))
    return out

